# revision 1
# baseline (speedup 1.0000x reference)
"""Trainium2 Bass kernel for chunked causal attention with static-routed LoRA experts.

Problem (hardcoded shapes): x [2, 4096, 768] f32; Wqkv [2304, 768]; Aqkv [8, 64, 768];
Bqkv [8, 3, 768, 64]; Wproj [768, 768]; Aproj [8, 64, 768]; Bproj [8, 768, 64];
expert_indices [4] i64; chunk_sizes [4] i64 (static routing metadata).

Sharding: 8 cores = 2 batches x 4 head-blocks (3 heads each). Each core computes its
heads' attention for all chunks and a partial output projection over its head columns;
host sums the 4 partials per batch (out-proj is linear in o) and transposes.

Layouts (all feature-on-partition except v):
  q_h, k_h: [64(d), n] sbuf bf16  -> QK matmul directly: scores[kv_tile, q] psum
  v_h:      [128(tokens), kv_tile, 65] bf16 ("v | ones" column for softmax row-sums)
  exp(scores) -> sbuf bf16 [kv_tile, q]  (ACT, no max subtraction: |scores| <~ 4)
  AV accumulates o_unnorm[65, q] over kv tiles; row 64 = softmax denominators.
  normalize via reciprocal + K=1 ones broadcast-matmul + DVE multiply.

Emission order: all chunks' projections first (PE-heavy), then per-chunk
attention + output projection (ACT-heavy) so the engines overlap.
"""

import numpy as np
import ml_dtypes

import concourse.bass as bass
import concourse.tile as tile
from concourse import bacc, mybir
from concourse import bass_utils

BF16 = mybir.dt.bfloat16
F32 = mybir.dt.float32
bf16 = ml_dtypes.bfloat16

DIM = 768
HEADS = 12
HD = 64
R = 64
LORA_SCALE = 2.0
QSCALE = HD ** -0.5
KT = DIM // 128  # 6 K-tiles over the model dim
B = 2
NCORES = 8
HPC = 3  # heads per core
NQ = 512  # matmul moving free dim (one psum bank of fp32)

_compiled_cache = {}


class _Ctx:
    pass


def _emit_loads(g):
    nc = g.nc
    nc.sync.dma_start(out=g.x_sb, in_=g.xT.rearrange("(kt p) n -> p kt n", p=128))
    nc.sync.dma_start(out=g.wqk_sb, in_=g.wqk.rearrange("(kt p) m -> p kt m", p=128))
    nc.sync.dma_start(out=g.wv_sb, in_=g.wv.rearrange("(kt p) m -> p kt m", p=128))
    nc.sync.dma_start(out=g.aq_sb, in_=g.aq.rearrange("c (kt p) r -> p c kt r", p=128))
    nc.sync.dma_start(out=g.bq_sb, in_=g.bq.rearrange("c p m -> p c m"))
    nc.sync.dma_start(out=g.bv_sb, in_=g.bv.rearrange("c p m -> p c m"))
    nc.sync.dma_start(out=g.wp0_sb, in_=g.wp0)
    nc.sync.dma_start(out=g.wp1_sb, in_=g.wp1)
    nc.sync.dma_start(out=g.ap0_sb, in_=g.ap0.rearrange("c p r -> p c r"))
    nc.sync.dma_start(out=g.ap1_sb, in_=g.ap1.rearrange("c p r -> p c r"))
    nc.sync.dma_start(out=g.bp_sb, in_=g.bp.rearrange("c p m -> p c m"))


def _emit_proj(g, c):
    """QKV projections (+LoRA) for chunk c -> q_sb/k_sb/v_sb."""
    nc = g.nc
    n_c = int(g.sizes[c])
    o0 = int(g.offs[c])
    ncols = slice(o0, o0 + n_c)
    nqh = [(j, min(NQ, n_c - j)) for j in range(0, n_c, NQ)]

    # LoRA down-projection h = A_e @ x_c^T : [r, n_c]
    h_ps = g.psacc.tile([64, n_c], F32, tag="acc")
    for (j, w) in nqh:
        for kt in range(KT):
            nc.tensor.matmul(
                h_ps[:, j:j + w],
                g.aq_sb[:, c, kt, :],
                g.x_sb[:, kt, o0 + j:o0 + j + w],
                start=(kt == 0), stop=(kt == KT - 1),
            )
    h_sb = g.hpool.tile([64, n_c], BF16, tag="h")
    nc.vector.tensor_copy(out=h_sb, in_=h_ps)

    # feature-layout q/k projection (M-tiles: [q0 q1][q2 k0][k1 k2])
    for mt in range(3):
        qk_ps = g.psbig.tile([128, n_c], F32, tag="big")
        for (j, w) in nqh:
            for kt in range(KT):
                nc.tensor.matmul(
                    qk_ps[:, j:j + w],
                    g.wqk_sb[:, kt, mt * 128:(mt + 1) * 128],
                    g.x_sb[:, kt, o0 + j:o0 + j + w],
                    start=(kt == 0), stop=False,
                )
            nc.tensor.matmul(
                qk_ps[:, j:j + w],
                g.bq_sb[:, c, mt * 128:(mt + 1) * 128],
                h_sb[:, j:j + w],
                start=False, stop=True,
            )
        for half in range(2):
            src = qk_ps[half * 64:(half + 1) * 64, :]
            idx = mt * 2 + half  # 0..5 = q0 q1 q2 k0 k1 k2
            if idx < 3:
                if g.qk_pack:
                    # duplicate q into both partition halves (rhs bp must
                    # match the packed lhsT bp)
                    nc.vector.tensor_copy(out=g.q_sb[0:64, idx, ncols], in_=src)
                    nc.vector.tensor_copy(out=g.q_sb[64:128, idx, ncols], in_=src)
                else:
                    nc.vector.tensor_copy(out=g.q_sb[:, idx, ncols], in_=src)
            else:
                if g.qk_pack:
                    # split k into even half (rows 0:64) / odd half (rows 64:128)
                    hcols = slice(o0 // 2, o0 // 2 + n_c // 2)
                    nc.vector.tensor_copy(
                        out=g.k_sb[0:64, idx - 3, hcols], in_=src[:, 0:n_c // 2])
                    nc.vector.tensor_copy(
                        out=g.k_sb[64:128, idx - 3, hcols], in_=src[:, n_c // 2:n_c])
                else:
                    nc.vector.tensor_copy(out=g.k_sb[:, idx - 3, ncols], in_=src)

    # token-layout v projection (+ LoRA delta), per 128-token tile
    for t in range(n_c // 128):
        v_ps = g.psbig.tile([128, 192], F32, tag="big")
        for kt in range(KT):
            nc.tensor.matmul(
                v_ps,
                g.x_sb[:, kt, o0 + t * 128:o0 + (t + 1) * 128],
                g.wv_sb[:, kt, :],
                start=(kt == 0), stop=False,
            )
        nc.tensor.matmul(
            v_ps,
            h_sb[:, t * 128:(t + 1) * 128],
            g.bv_sb[:, c, :],
            start=False, stop=True,
        )
        gt = o0 // 128 + t
        for h in range(HPC):
            nc.vector.tensor_copy(
                out=g.v_sb[:, h, gt, 0:64], in_=v_ps[:, h * 64:(h + 1) * 64]
            )
    return h_sb


def _emit_attn_outproj(g, c):
    _emit_attn_norm(g, c)
    _emit_outproj(g, c)


def _emit_attn_norm(g, c):
    """Attention over kv chunks 0..c + softmax normalization, for chunk c."""
    nc = g.nc
    n_c = int(g.sizes[c])
    o0 = int(g.offs[c])
    nqh = [(j, min(NQ, n_c - j)) for j in range(0, n_c, NQ)]

    on01 = g.onorm.tile([128, n_c], BF16, tag="on01")
    on2 = g.onorm.tile([64, n_c], BF16, tag="on2")
    g.on_tiles[c] = (on01, on2)
    kvt_end = int(g.offs[c + 1]) // 128
    for h in range(HPC):
        o_ps = g.psacc.tile([65, n_c], F32, tag="acc")
        if g.qk_pack:
            # (global tile index, k_sb row half, k_sb col tile index)
            kv_iter = []
            for cc in range(c + 1):
                nt = int(g.sizes[cc]) // 128
                half = nt // 2
                g0 = int(g.offs[cc]) // 128
                c0 = int(g.offs[cc]) // 256
                for i in range(half):
                    kv_iter.append(((g0 + i, 0, c0 + i), (g0 + half + i, 1, c0 + i)))
        else:
            kv_iter = [((t, None, None),) for t in range(kvt_end)]
        n_av = kvt_end
        avi = 0
        for pair in kv_iter:
            exps = []
            for (gt, rhalf, kcol) in pair:
                sc_ps = g.psbig.tile([128, n_c], F32, tag="big")
                for (j, w) in nqh:
                    if g.qk_pack:
                        lhsT = g.k_sb[rhalf * 64:(rhalf + 1) * 64, h,
                                      kcol * 128:(kcol + 1) * 128]
                        rhs = g.q_sb[rhalf * 64:(rhalf + 1) * 64, h,
                                     o0 + j:o0 + j + w]
                    else:
                        lhsT = g.k_sb[:, h, gt * 128:(gt + 1) * 128]
                        rhs = g.q_sb[:, h, o0 + j:o0 + j + w]
                    nc.tensor.matmul(
                        sc_ps[:, j:j + w], lhsT, rhs, start=True, stop=True,
                    )
                exp_sb = g.expool.tile([128, n_c], BF16, tag="exp")
                nc.scalar.activation(
                    out=exp_sb, in_=sc_ps, func=mybir.ActivationFunctionType.Exp
                )
                exps.append((gt, exp_sb))
            for (gt, exp_sb) in exps:
                for (j, w) in nqh:
                    nc.tensor.matmul(
                        o_ps[:, j:j + w],
                        g.v_sb[:, h, gt, :],
                        exp_sb[:, j:j + w],
                        start=(avi == 0), stop=(avi == n_av - 1),
                    )
                avi += 1
        rs_sb = g.onorm.tile([1, n_c], F32, tag="rs")
        nc.vector.reciprocal(out=rs_sb, in_=o_ps[64:65, :])
        bc_ps = g.psbig.tile([64, n_c], F32, tag="big")
        for (j, w) in nqh:
            nc.tensor.matmul(
                bc_ps[:, j:j + w], g.ones_sb, rs_sb[:, j:j + w],
                start=True, stop=True,
            )
        bc_sb = g.onorm.tile([64, n_c], F32, tag="bc")
        nc.vector.tensor_copy(out=bc_sb, in_=bc_ps)
        dst = on01[0:64, :] if h == 0 else (on01[64:128, :] if h == 1 else on2)
        nc.vector.tensor_mul(dst, o_ps[0:64, :], bc_sb)


def _emit_outproj(g, c):
    """Partial output projection for chunk c (reads on01/on2 from attn phase)."""
    nc = g.nc
    n_c = int(g.sizes[c])
    o0 = int(g.offs[c])
    ncols = slice(o0, o0 + n_c)
    nqh = [(j, min(NQ, n_c - j)) for j in range(0, n_c, NQ)]
    on01, on2 = g.on_tiles.pop(c)

    optag = "acc" if g.op_in_acc else "big"
    oppool = g.psacc if g.op_in_acc else g.psbig

    h2_ps = g.psacc.tile([64, n_c], F32, tag="acc")
    for (j, w) in nqh:
        nc.tensor.matmul(
            h2_ps[:, j:j + w], g.ap0_sb[:, c, :], on01[:, j:j + w],
            start=True, stop=False,
        )
        nc.tensor.matmul(
            h2_ps[:, j:j + w], g.ap1_sb[:, c, :], on2[:, j:j + w],
            start=False, stop=True,
        )
    h2_sb = g.outpool.tile([64, n_c], BF16, tag="h2")
    nc.vector.tensor_copy(out=h2_sb, in_=h2_ps)
    for mt in range(KT):
        op_ps = oppool.tile([128, n_c], F32, tag=optag)
        for (j, w) in nqh:
            nc.tensor.matmul(
                op_ps[:, j:j + w],
                g.wp0_sb[:, mt * 128:(mt + 1) * 128],
                on01[:, j:j + w],
                start=True, stop=False,
            )
            nc.tensor.matmul(
                op_ps[:, j:j + w],
                g.wp1_sb[:, mt * 128:(mt + 1) * 128],
                on2[:, j:j + w],
                start=False, stop=False,
            )
            nc.tensor.matmul(
                op_ps[:, j:j + w],
                g.bp_sb[:, c, mt * 128:(mt + 1) * 128],
                h2_sb[:, j:j + w],
                start=False, stop=True,
            )
        out_sb = g.outpool.tile([128, n_c], F32, tag="osb")
        nc.vector.tensor_copy(out=out_sb, in_=op_ps)
        nc.sync.dma_start(out=g.out[mt * 128:(mt + 1) * 128, ncols], in_=out_sb)


def _build_program(sizes, n_total, iters=1, order="serial", qk_pack=False):
    """Build + compile the per-core Bass program (same program for all 8 cores).

    iters > 1 repeats the whole computation (including input DMA loads) for
    on-device timing via wall-clock deltas; results are identical.
    order: emission pattern ("serial" | "lookahead" | "allproj").
    qk_pack: row-pack QK matmul pairs (K=64 each) into array halves.
    """
    C = len(sizes)
    offs = np.concatenate([[0], np.cumsum(sizes)]).astype(int)
    if qk_pack:
        assert all((int(s) // 128) % 2 == 0 for s in sizes), "qk_pack needs even tile counts"

    nc = bacc.Bacc("TRN2", target_bir_lowering=False, debug=False)
    g = _Ctx()
    g.nc = nc
    g.sizes = sizes
    g.offs = offs
    g.qk_pack = qk_pack
    g.on_tiles = {}
    g.op_in_acc = False

    def din(name, shape, dt=BF16):
        ap = nc.dram_tensor(name, list(shape), dt, kind="ExternalInput").ap()
        setattr(g, name, ap)

    din("xT", [DIM, n_total])
    din("wqk", [DIM, 384])
    din("wv", [DIM, 192])
    din("aq", [C, DIM, R])
    din("bq", [C, R, 384])
    din("bv", [C, R, 192])
    din("wp0", [128, DIM])
    din("wp1", [64, DIM])
    din("ap0", [C, 128, R])
    din("ap1", [C, 64, R])
    din("bp", [C, R, DIM])
    g.out = nc.dram_tensor("out", [DIM, n_total], F32, kind="ExternalOutput").ap()

    KVT_TOT = n_total // 128

    with tile.TileContext(nc) as tc:
        with (
            tc.tile_pool(name="singles", bufs=1) as singles,
            tc.tile_pool(name="hpool", bufs=4) as hpool,
            tc.tile_pool(name="expool", bufs=6) as expool,
            tc.tile_pool(name="onorm", bufs=2) as onorm,
            tc.tile_pool(name="outpool", bufs=3) as outpool,
            tc.tile_pool(name="psbig", bufs=2, space="PSUM") as psbig,
            tc.tile_pool(name="psacc", bufs=2, space="PSUM") as psacc,
        ):
            g.hpool, g.expool, g.onorm = hpool, expool, onorm
            g.outpool, g.psbig, g.psacc = outpool, psbig, psacc

            # persistent SBUF tiles
            g.x_sb = singles.tile([128, KT, n_total], BF16)
            g.wqk_sb = singles.tile([128, KT, 384], BF16)
            g.wv_sb = singles.tile([128, KT, 192], BF16)
            g.aq_sb = singles.tile([128, C, KT, R], BF16)
            g.bq_sb = singles.tile([64, C, 384], BF16)
            g.bv_sb = singles.tile([64, C, 192], BF16)
            g.wp0_sb = singles.tile([128, DIM], BF16)
            g.wp1_sb = singles.tile([64, DIM], BF16)
            g.ap0_sb = singles.tile([128, C, R], BF16)
            g.ap1_sb = singles.tile([64, C, R], BF16)
            g.bp_sb = singles.tile([64, C, DIM], BF16)

            g.ones_sb = singles.tile([1, 64], F32)
            nc.vector.memset(g.ones_sb, 1.0)

            if qk_pack:
                g.q_sb = singles.tile([128, HPC, n_total], BF16)
                g.k_sb = singles.tile([128, HPC, n_total // 2], BF16)
            else:
                g.q_sb = singles.tile([64, HPC, n_total], BF16)
                g.k_sb = singles.tile([64, HPC, n_total], BF16)
            g.v_sb = singles.tile([128, HPC, KVT_TOT, 65], BF16)
            nc.vector.memset(g.v_sb[:, :, :, 64:65], 1.0)

            for _it in range(iters):
                _emit_loads(g)
                if order == "allproj":
                    for c in range(C):
                        _emit_proj(g, c)
                    for c in range(C):
                        _emit_attn_outproj(g, c)
                elif order == "lookahead":
                    # P0 P1 A0 P2 A1 P3 A2 A3
                    _emit_proj(g, 0)
                    for c in range(C):
                        if c + 1 < C:
                            _emit_proj(g, c + 1)
                        _emit_attn_outproj(g, c)
                else:  # "serial": P0 A0 P1 A1 ...
                    for c in range(C):
                        _emit_proj(g, c)
                        _emit_attn_outproj(g, c)

    nc.compile()
    return nc


def _prep_core_inputs(core, x, Wqkv, Aqkv, Bqkv, Wproj, Aproj, Bproj, eidx, sizes):
    b = core // 4
    hblk = core % 4
    hs = [HPC * hblk + j for j in range(HPC)]
    C = len(sizes)

    def cast(a):
        return np.ascontiguousarray(a).astype(bf16)

    xT = cast(x[b].T)
    # base qkv slices; q scaled by QSCALE; M order [q0 q1 q2 k0 k1 k2]
    wqk = cast(np.concatenate(
        [QSCALE * Wqkv[64 * h:64 * h + 64].T for h in hs]
        + [Wqkv[DIM + 64 * h:DIM + 64 * h + 64].T for h in hs], axis=1))
    wv = cast(np.concatenate(
        [Wqkv[2 * DIM + 64 * h:2 * DIM + 64 * h + 64].T for h in hs], axis=1))
    aq = cast(np.stack([Aqkv[eidx[c]].T for c in range(C)]))
    bq = cast(np.stack([
        np.concatenate(
            [QSCALE * LORA_SCALE * Bqkv[eidx[c], 0, 64 * h:64 * h + 64].T for h in hs]
            + [LORA_SCALE * Bqkv[eidx[c], 1, 64 * h:64 * h + 64].T for h in hs], axis=1)
        for c in range(C)]))
    bv = cast(np.stack([
        np.concatenate(
            [LORA_SCALE * Bqkv[eidx[c], 2, 64 * h:64 * h + 64].T for h in hs], axis=1)
        for c in range(C)]))
    col0 = 192 * hblk
    wpT = Wproj[:, col0:col0 + 192].T  # [192, 768]
    wp0 = cast(wpT[0:128])
    wp1 = cast(wpT[128:192])
    apT = np.stack([Aproj[eidx[c]][:, col0:col0 + 192].T for c in range(C)])  # [C,192,64]
    ap0 = cast(apT[:, 0:128])
    ap1 = cast(apT[:, 128:192])
    bpv = cast(np.stack([LORA_SCALE * Bproj[eidx[c]].T for c in range(C)]))
    return {
        "xT": xT, "wqk": wqk, "wv": wv, "aq": aq, "bq": bq, "bv": bv,
        "wp0": wp0, "wp1": wp1, "ap0": ap0, "ap1": ap1, "bp": bpv,
    }


def kernel(x, Wqkv, Aqkv, Bqkv, Wproj, Aproj, Bproj, expert_indices, chunk_sizes):
    x = np.asarray(x, dtype=np.float32)
    Wqkv = np.asarray(Wqkv, dtype=np.float32)
    Aqkv = np.asarray(Aqkv, dtype=np.float32)
    Bqkv = np.asarray(Bqkv, dtype=np.float32)
    Wproj = np.asarray(Wproj, dtype=np.float32)
    Aproj = np.asarray(Aproj, dtype=np.float32)
    Bproj = np.asarray(Bproj, dtype=np.float32)
    sizes = [int(s) for s in np.asarray(chunk_sizes)]
    eidx = [int(e) for e in np.asarray(expert_indices)]
    n_total = int(sum(sizes))
    assert x.shape == (B, n_total, DIM)
    assert all(s % 128 == 0 for s in sizes)

    key = (tuple(sizes), n_total)
    if key not in _compiled_cache:
        _compiled_cache[key] = _build_program(sizes, n_total)
    nc = _compiled_cache[key]

    in_maps = [
        _prep_core_inputs(c, x, Wqkv, Aqkv, Bqkv, Wproj, Aproj, Bproj, eidx, sizes)
        for c in range(NCORES)
    ]
    res = bass_utils.run_bass_kernel_spmd(nc, in_maps, core_ids=list(range(NCORES)))
    outp = np.empty((B, n_total, DIM), dtype=np.float32)
    for b in range(B):
        acc = res.results[4 * b]["out"].astype(np.float32)
        for j in range(1, 4):
            acc = acc + res.results[4 * b + j]["out"]
        outp[b] = acc.T
    return outp
